# revision 1
# baseline (speedup 1.0000x reference)
"""FAISS-anchor kernel layer on 8 Trainium2 NeuronCores (Bass/Tile).

Problem (per full input):
    x [8,8192,3], Key [1024,3], init_mat/Value [1024,256],
    w1 [3,1024], b1 [1024], w2 [1024,256], b2 [256]
    idx = argmin_a ||x - Key_a||^2           (exact 1-NN, first-tie)
    out = gelu((x - Key[idx]) @ w1 + b1) @ w2 + b2 + (init_mat + Value)[idx]

Sharding: pure data-parallel — core c takes batch element c (8192 tokens).
All tables (Key-derived features, V-table, MLP weights) are replicated.

Device pipeline per 128-token tile:
    PE:   s = -||x-k||^2 for all 1024 anchors (two row-group-packed f32r
          matmuls with [2x, -|x|^2, -1] x [k, 1, |k|^2] feature folding)
    DVE:  max8 -> top-8 of s; max_index -> anchor index (first-tie == argmin)
    DMA:  indirect gather of fused table row [V+init+b2 | 2*Key] per token
    GPS:  ret2 = 2x - 2*Key[idx]
    PE:   transpose ret2 -> [4, tok]; h^T = (0.5*w1)^T @ ret2^T (K=4)
    ACT:  gelu(h^T + b1)  (exact erf gelu)
    PE:   out = h @ w2 (hT tiles as stationary operand, K=8x128)
    ACT+GPS: PSUM->SBUF copy, += gathered V row; DMA out.

Host: packs layouts, runs 8 cores via run_bass_kernel_spmd, re-assembles,
and re-resolves near-tie tokens (top-2 gap below tau) with exact fp32
reference arithmetic so fp32r matmul rounding cannot flip the argmin.
"""

import numpy as np

B, N, A, D_IN, D_OUT = 8, 8192, 1024, 3, 256
H = 4 * D_OUT
P = 128
NT = N // P            # 64 token tiles per core
TPC = 8                # tiles per chunk
NCHUNK = NT // TPC     # 8 chunks
VT_W = 320             # gather-table row width (256 V + 3 key + pad), 1280B
N_CORES = 8

_PROGRAM = None  # (nc, input_names)


def _build_program():
    import concourse.bass as bass
    import concourse.mybir as mybir
    import concourse.tile as tile
    from concourse import bacc

    f32 = mybir.dt.float32
    f32r = mybir.dt.float32r
    u32 = mybir.dt.uint32

    # Bacc (not raw Bass): its compile() splits multi-sem waits and moves
    # matmul waits onto ldweights — TRN2 allows at most 1 wait per instr.
    nc = bacc.Bacc("TRN2", target_bir_lowering=False, debug=False)

    # DRAM I/O
    xh_d = nc.dram_tensor("xh", [P, N], f32r, kind="ExternalInput").ap()
    kh_d = nc.dram_tensor("kh", [P, A], f32r, kind="ExternalInput").ap()
    x2_d = nc.dram_tensor("x2", [P, NT * 4], f32, kind="ExternalInput").ap()
    w1h_d = nc.dram_tensor("w1h", [4, H], f32r, kind="ExternalInput").ap()
    b1p_d = nc.dram_tensor("b1p", [P, H // P], f32, kind="ExternalInput").ap()
    w2p_d = nc.dram_tensor("w2p", [P, (H // P) * D_OUT], f32r, kind="ExternalInput").ap()
    vt_d = nc.dram_tensor("vt", [A, VT_W], f32, kind="ExternalInput").ap()
    id_d = nc.dram_tensor("ident", [P, P], f32, kind="ExternalInput").ap()

    out_d = nc.dram_tensor("outp", [N, D_OUT], f32, kind="ExternalOutput").ap()
    m8_d = nc.dram_tensor("m8o", [P, NT * 8], f32, kind="ExternalOutput").ap()
    idx_d = nc.dram_tensor("idxo", [P, NT * 8], u32, kind="ExternalOutput").ap()

    HC = H // P  # 8 H-chunks

    with tile.TileContext(nc) as tc:
        with (
            tc.tile_pool(name="const", bufs=1) as cpool,
            tc.tile_pool(name="xh", bufs=2) as xhpool,
            tc.tile_pool(name="vg", bufs=16) as vgpool,
            tc.tile_pool(name="rl", bufs=4) as rlpool,
            tc.tile_pool(name="rlts", bufs=2) as rltspool,
            tc.tile_pool(name="ht", bufs=16) as htpool,
            tc.tile_pool(name="m8", bufs=1) as m8pool,
            tc.tile_pool(name="idx", bufs=2) as idxpool,
            tc.tile_pool(name="ob", bufs=3) as obpool,
            tc.tile_pool(name="spsum", bufs=2, space="PSUM") as spsum,
            tc.tile_pool(name="hpsum", bufs=2, space="PSUM") as hpsum,
            tc.tile_pool(name="rpsum", bufs=1, space="PSUM") as rpsum,
            tc.tile_pool(name="opsum", bufs=1, space="PSUM") as opsum,
        ):
            # Resident constants
            kh_t = cpool.tile([P, A], f32r)
            nc.sync.dma_start(out=kh_t[:], in_=kh_d[:])
            x2_t = cpool.tile([P, NT, 4], f32)
            nc.sync.dma_start(out=x2_t[:], in_=x2_d[:])
            w1h_t = cpool.tile([4, H], f32r)
            nc.sync.dma_start(out=w1h_t[:], in_=w1h_d[:])
            b1p_t = cpool.tile([P, HC], f32)
            nc.sync.dma_start(out=b1p_t[:], in_=b1p_d[:])
            w2p_t = cpool.tile([P, HC * D_OUT], f32r)
            nc.sync.dma_start(out=w2p_t[:], in_=w2p_d[:])
            id_t = cpool.tile([P, P], f32)
            nc.sync.dma_start(out=id_t[:], in_=id_d[:])
            m8_t = m8pool.tile([P, NT * 8], f32)

            for c in range(NCHUNK):
                # lhsT features for this chunk's 8 token tiles (rows 0-4 and
                # a replica at rows 32-36 for 2-way row-group packing).
                xh_t = xhpool.tile([P, TPC * P], f32r)
                nc.sync.dma_start(
                    out=xh_t[:], in_=xh_d[:, c * TPC * P : (c + 1) * TPC * P]
                )
                idx_t = idxpool.tile([P, TPC, 8], u32)

                for j in range(TPC):
                    t = c * TPC + j
                    s_ps = spsum.tile([P, A], f32, tag="s")
                    for g in range(2):  # two 512-anchor halves, packed rows
                        nc.tensor.matmul(
                            out=s_ps[:, g * 512 : (g + 1) * 512],
                            lhsT=xh_t[
                                32 * g : 32 * g + 5, j * P : (j + 1) * P
                            ],
                            rhs=kh_t[32 * g : 32 * g + 5, g * 512 : (g + 1) * 512],
                            start=True,
                            stop=True,
                            tile_position=(32 * g, 0),
                        )
                    m8 = m8_t[:, t * 8 : (t + 1) * 8]
                    nc.vector.max(m8, s_ps[:])
                    nc.vector.max_index(idx_t[:, j, :], m8, s_ps[:])

                # Gather fused table rows for the chunk's 1024 tokens.
                # NB: one offset per partition ([P,1]) per call into an
                # offset-0 [P, W] dest tile — both multi-index offsets and
                # non-zero dest offsets are mishandled by the real SWDGE
                # (CoreSim accepts them but hardware does not).
                vg_ts = []
                for j in range(TPC):
                    vg_j = vgpool.tile([P, VT_W], f32, tag="vg")
                    nc.gpsimd.indirect_dma_start(
                        out=vg_j[:],
                        out_offset=None,
                        in_=vt_d[:],
                        in_offset=bass.IndirectOffsetOnAxis(
                            ap=idx_t[:, j, 0:1], axis=0
                        ),
                    )
                    vg_ts.append(vg_j)
                nc.sync.dma_start(
                    out=idx_d[:, c * TPC * 8 : (c + 1) * TPC * 8],
                    in_=idx_t[:],
                )

                for half in range(2):
                    rlt_ps = rpsum.tile([4, 512], f32, tag="rlt")
                    for q in range(4):
                        j = half * 4 + q
                        rl = rlpool.tile([P, 4], f32, tag="rl")
                        nc.gpsimd.tensor_tensor(
                            out=rl[:],
                            in0=x2_t[:, c * TPC + j, :],
                            in1=vg_ts[j][:, D_OUT : D_OUT + 4],
                            op=mybir.AluOpType.subtract,
                        )
                        nc.tensor.transpose(
                            out=rlt_ps[:, q * P : (q + 1) * P],
                            in_=rl[:],
                            identity=id_t[:],
                        )
                    rlts = rltspool.tile([4, 512], f32r)
                    nc.scalar.activation(
                        out=rlts[:],
                        in_=rlt_ps[:],
                        func=mybir.ActivationFunctionType.Copy,
                    )

                    hts = []
                    for hc in range(HC):
                        h_ps = hpsum.tile([P, 512], f32, tag="h")
                        nc.tensor.matmul(
                            out=h_ps[:],
                            lhsT=w1h_t[:, hc * P : (hc + 1) * P],
                            rhs=rlts[:],
                            start=True,
                            stop=True,
                        )
                        ht = htpool.tile([P, 512], f32r, tag="ht")
                        nc.scalar.activation(
                            out=ht[:],
                            in_=h_ps[:],
                            func=mybir.ActivationFunctionType.Gelu,
                            bias=b1p_t[:, hc : hc + 1],
                        )
                        hts.append(ht)

                    for q in range(4):
                        j = half * 4 + q
                        t = c * TPC + j
                        o_ps = opsum.tile([P, D_OUT], f32, tag="o")
                        for hc in range(HC):
                            nc.tensor.matmul(
                                out=o_ps[:],
                                lhsT=hts[hc][:, q * P : (q + 1) * P],
                                rhs=w2p_t[:, hc * D_OUT : (hc + 1) * D_OUT],
                                start=(hc == 0),
                                stop=(hc == HC - 1),
                            )
                        ob = obpool.tile([P, D_OUT], f32)
                        nc.scalar.activation(
                            out=ob[:],
                            in_=o_ps[:],
                            func=mybir.ActivationFunctionType.Copy,
                        )
                        nc.gpsimd.tensor_tensor(
                            out=ob[:],
                            in0=ob[:],
                            in1=vg_ts[j][:, 0:D_OUT],
                            op=mybir.AluOpType.add,
                        )
                        nc.sync.dma_start(
                            out=out_d[t * P : (t + 1) * P, :], in_=ob[:]
                        )

            nc.sync.dma_start(out=m8_d[:], in_=m8_t[:])

    nc.compile()
    names = ["xh", "kh", "x2", "w1h", "b1p", "w2p", "vt", "ident"]
    return nc, names


def _get_program():
    global _PROGRAM
    if _PROGRAM is None:
        _PROGRAM = _build_program()
    return _PROGRAM


def _host_pack(x, Key, init_mat, Value, w1, b1, w2, b2):
    """Build per-core input dicts (host-side layout packing)."""
    f = np.float32
    Key = np.asarray(Key, f)
    x = np.asarray(x, f)
    k2 = np.sum(Key * Key, axis=1)  # [A]

    # khat rows: [k0,k1,k2,1,|k|^2]; s = 2x.k - |x|^2 - |k|^2 = -d2
    kh = np.zeros((P, A), f)
    kf = np.concatenate([Key, np.ones((A, 1), f), k2[:, None]], axis=1)  # [A,5]
    for g in range(4):
        kh[32 * g : 32 * g + 5, :] = kf.T

    w1h = np.zeros((4, H), f)
    w1h[:3, :] = 0.5 * np.asarray(w1, f)
    b1p = np.asarray(b1, f).reshape(H // P, P).T.copy()  # [128, 8]
    w2p = (
        np.asarray(w2, f)
        .reshape(H // P, P, D_OUT)
        .transpose(1, 0, 2)
        .reshape(P, (H // P) * D_OUT)
        .copy()
    )
    vt = np.zeros((A, VT_W), f)
    vt[:, :D_OUT] = np.asarray(init_mat, f) + np.asarray(Value, f) + np.asarray(b2, f)
    vt[:, D_OUT : D_OUT + 3] = 2.0 * Key
    ident = np.eye(P, dtype=f)

    in_maps = []
    for c in range(N_CORES):
        xc = x[c]  # [N, 3]
        x2sq = np.sum(xc * xc, axis=1)  # [N]
        # xhat features [N, 5]: [2x, -|x|^2, -1]
        xf = np.concatenate(
            [2.0 * xc, -x2sq[:, None], -np.ones((N, 1), f)], axis=1
        ).astype(f)
        # packed lhsT [128, N]: tile t at cols t*128..; features at rows 0-4
        # plus a replica at rows 32-36 for the second row-group.
        xh = np.zeros((P, N), f)
        xf_t = xf.reshape(NT, P, 5).transpose(2, 0, 1).reshape(5, N)  # [5, NT*P]
        xh[0:5, :] = xf_t
        xh[32:37, :] = xf_t

        x2q = np.zeros((N, 4), f)
        x2q[:, :3] = 2.0 * xc
        x2 = x2q.reshape(NT, P, 4).transpose(1, 0, 2).reshape(P, NT * 4).copy()

        in_maps.append(
            {
                "xh": xh,
                "kh": kh,
                "x2": x2,
                "w1h": w1h,
                "b1p": b1p,
                "w2p": w2p,
                "vt": vt,
                "ident": ident,
            }
        )
    return in_maps


def _erf(z):
    # Abramowitz-Stegun is not enough; use the exact erf from scipy if
    # present, else jax (available wherever the bass stack runs).
    try:
        from scipy.special import erf

        return erf(z)
    except ImportError:
        import jax

        with jax.default_device(jax.devices("cpu")[0]):
            return np.asarray(jax.scipy.special.erf(np.asarray(z, np.float32)))


def _refine(out, m8o, idxo, x, Key, init_mat, Value, w1, b1, w2, b2, tau=0.03):
    """Re-resolve tokens whose top-2 score gap is within tau (near-ties):
    recompute their argmin + output row in exact fp32 reference arithmetic."""
    f = np.float32
    Key = np.asarray(Key, f)
    V = np.asarray(init_mat, f) + np.asarray(Value, f)
    k2 = np.sum(Key * Key, axis=1)
    n_fixed = 0
    for c in range(out.shape[0]):
        m8 = m8o[c]  # [128, NT*8]
        m0 = m8[:, 0::8]  # [128, NT]
        m1 = m8[:, 1::8]
        gap = m0 - m1  # s-space gap == d2 second - d2 min
        dev_idx = idxo[c][:, 0::8].astype(np.int64)  # [128, NT]
        scale = 1.0 + np.abs(m0)
        flag = gap < tau * scale  # [128, NT]
        ps, ts = np.nonzero(flag)
        if ps.size == 0:
            continue
        toks = ts * P + ps
        xc = np.asarray(x[c], f)[toks]  # [F, 3]
        d2 = -2.0 * (xc @ Key.T) + k2[None, :]  # reference formula, fp32
        amin = np.argmin(d2, axis=1)
        mism = amin != dev_idx[ps, ts]
        if not np.any(mism):
            continue
        toks = toks[mism]
        amin = amin[mism]
        xe = np.asarray(x[c], f)[toks]
        rl = xe - Key[amin]
        pre = (rl @ np.asarray(w1, f) + np.asarray(b1, f)).astype(f)
        h = (0.5 * pre * (1.0 + _erf(pre / np.sqrt(f(2.0))))).astype(f)
        row = (h @ np.asarray(w2, f) + np.asarray(b2, f) + V[amin]).astype(f)
        out[c, toks, :] = row
        n_fixed += toks.size
    return n_fixed


def kernel(**inputs):
    from concourse.bass_utils import run_bass_kernel_spmd

    nc, names = _get_program()
    in_maps = _host_pack(**inputs)
    res = run_bass_kernel_spmd(nc, in_maps, core_ids=list(range(N_CORES)))

    out = np.zeros((B, N, D_OUT), np.float32)
    m8o = np.zeros((B, P, NT * 8), np.float32)
    idxo = np.zeros((B, P, NT * 8), np.uint32)
    for c in range(N_CORES):
        r = res.results[c]
        out[c] = r["outp"]
        m8o[c] = r["m8o"]
        idxo[c] = r["idxo"]

    _refine(out, m8o, idxo, **inputs)
    return out


if __name__ == "__main__":
    # smoke: build only
    _get_program()
    print("program built")



# revision 3
# speedup vs baseline: 1.3676x; 1.3676x over previous
"""FAISS-anchor kernel layer on 8 Trainium2 NeuronCores (Bass/Tile).

Problem (per full input):
    x [8,8192,3], Key [1024,3], init_mat/Value [1024,256],
    w1 [3,1024], b1 [1024], w2 [1024,256], b2 [256]
    idx = argmin_a ||x - Key_a||^2           (exact 1-NN, first-tie)
    out = gelu((x - Key[idx]) @ w1 + b1) @ w2 + b2 + (init_mat + Value)[idx]

Sharding: pure data-parallel - core c takes batch element c (8192 tokens).
All tables (Key-derived features, V-table, MLP weights) are replicated.

v2 design (vs the f32r baseline):
  * fp16 PE operands everywhere (scores, w1, rl^T, gelu out, w2): PE streams
    16-bit at ~2.4x the f32r rate and fp16's 10-bit mantissa keeps the same
    near-tie refine threshold (tau=0.03) as f32r.
  * software pipeline: round r runs chunk r's score/argmax/gather stage
    against chunk r-1's MLP stage so PE never waits on the DVE->GPS chain.
  * engine re-balance: GPSIMD only does the indirect gather + tiny rl
    subtract; the V-add runs on DVE straight out of PSUM (kills the
    ACT copy + GPS add of the baseline and their semaphores).
  * PSUM: scores 2x2 banks, h 2x1, shared {rl^T, out} tag ring 2x1 = 8.

Device pipeline per 128-token tile:
    PE:   s = -||x-k||^2 for all 1024 anchors (two row-group-packed fp16
          matmuls with [2x, -|x|^2, -1] x [k, 1, |k|^2] feature folding)
    DVE:  max8 -> top-8 of s; max_index -> anchor index
    GPS:  indirect gather of fused table row [V+init+b2 | 2*Key] per token;
          rl2 = 2x - 2*Key[idx] (fp16 out)
    PE:   transpose rl2 -> [4, tok] (fp16); h^T = (0.5*w1)^T @ rl2^T (K=4)
    ACT:  rl^T PSUM->SBUF fp16 copy; gelu(h^T + b1) -> fp16
    PE:   out = h @ w2 (h^T tiles stationary, K=8x128, fp16)
    DVE:  ob = out_psum + gathered V row; paired-store DMA out.

Host: packs layouts, runs 8 cores via run_bass_kernel_spmd, re-assembles,
and re-resolves near-tie tokens (top-2 gap below tau) with exact fp32
reference arithmetic so fp16 matmul rounding cannot flip the argmin.
"""

import numpy as np

B, N, A, D_IN, D_OUT = 8, 8192, 1024, 3, 256
H = 4 * D_OUT
P = 128
NT = N // P            # 64 token tiles per core
TPC = 8                # tiles per chunk
NCHUNK = NT // TPC     # 8 chunks
VT_W = 264             # gather-table row width (256 V + 3 key + pad), 1056B
N_CORES = 8

_PROGRAM = None  # (nc, input_names)


def _build_program():
    import concourse.bass as bass
    import concourse.mybir as mybir
    import concourse.tile as tile
    from concourse import bacc

    f32 = mybir.dt.float32
    f16 = mybir.dt.float16
    u32 = mybir.dt.uint32

    # Bacc (not raw Bass): its compile() splits multi-sem waits and moves
    # matmul waits onto ldweights - TRN2 allows at most 1 wait per instr.
    nc = bacc.Bacc("TRN2", target_bir_lowering=False, debug=False)

    # DRAM I/O
    xh_d = nc.dram_tensor("xh", [5, N], f16, kind="ExternalInput").ap()
    kh_d = nc.dram_tensor("kh", [P, A], f16, kind="ExternalInput").ap()
    x2_d = nc.dram_tensor("x2", [P, NT * 4], f32, kind="ExternalInput").ap()
    w1h_d = nc.dram_tensor("w1h", [4, H], f16, kind="ExternalInput").ap()
    b1p_d = nc.dram_tensor("b1p", [P, H // P], f32, kind="ExternalInput").ap()
    w2p_d = nc.dram_tensor("w2p", [P, (H // P) * D_OUT], f16, kind="ExternalInput").ap()
    vt_d = nc.dram_tensor("vt", [A, VT_W], f32, kind="ExternalInput").ap()
    id_d = nc.dram_tensor("ident", [P, P], f16, kind="ExternalInput").ap()

    out_d = nc.dram_tensor("outp", [N, D_OUT], f32, kind="ExternalOutput").ap()
    m8_d = nc.dram_tensor("m8o", [P, NT * 8], f32, kind="ExternalOutput").ap()
    idx_d = nc.dram_tensor("idxo", [P, NT * 8], u32, kind="ExternalOutput").ap()

    HC = H // P  # 8 H-chunks

    with tile.TileContext(nc) as tc:
        with (
            tc.tile_pool(name="const", bufs=1) as cpool,
            tc.tile_pool(name="xh", bufs=2) as xhpool,
            tc.tile_pool(name="vg", bufs=16) as vgpool,
            tc.tile_pool(name="rl", bufs=4) as rlpool,
            tc.tile_pool(name="rlts", bufs=2) as rltspool,
            tc.tile_pool(name="ht", bufs=16) as htpool,
            tc.tile_pool(name="m8", bufs=1) as m8pool,
            tc.tile_pool(name="idx", bufs=2) as idxpool,
            tc.tile_pool(name="ob", bufs=3) as obpool,
            tc.tile_pool(name="spsum", bufs=2, space="PSUM") as spsum,
            tc.tile_pool(name="hpsum", bufs=2, space="PSUM") as hpsum,
            tc.tile_pool(name="ropsum", bufs=2, space="PSUM") as ropsum,
        ):
            # Resident constants
            kh_t = cpool.tile([P, A], f16)
            nc.sync.dma_start(out=kh_t[:], in_=kh_d[:])
            x2_t = cpool.tile([P, NT, 4], f32)
            nc.sync.dma_start(out=x2_t[:], in_=x2_d[:])
            w1h_t = cpool.tile([4, H], f16)
            nc.sync.dma_start(out=w1h_t[:], in_=w1h_d[:])
            b1p_t = cpool.tile([P, HC], f32)
            nc.sync.dma_start(out=b1p_t[:], in_=b1p_d[:])
            w2p_t = cpool.tile([P, HC * D_OUT], f16)
            nc.sync.dma_start(out=w2p_t[:], in_=w2p_d[:])
            id_t = cpool.tile([P, P], f16)
            nc.sync.dma_start(out=id_t[:], in_=id_d[:])
            m8_t = m8pool.tile([P, NT * 8], f32)

            # xh tiles: features at partition rows 0-4 plus a replica at
            # rows 32-36 for 2-way PE row-group packing.
            def load_xh(c):
                xh_t = xhpool.tile([37, TPC * P], f16, tag="xh")
                sl = xh_d[:, c * TPC * P : (c + 1) * TPC * P]
                nc.sync.dma_start(out=xh_t[0:5, :], in_=sl)
                nc.sync.dma_start(out=xh_t[32:37, :], in_=sl)
                return xh_t

            xh_t = load_xh(0)
            vg_all = [None] * NCHUNK  # per-chunk gathered rows, live 1 round

            for r in range(NCHUNK + 1):
                cs, cm = r, r - 1

                # --- score/argmax stage for chunk cs -------------------
                if cs < NCHUNK:
                    cur_xh = xh_t
                    if cs + 1 < NCHUNK:
                        xh_t = load_xh(cs + 1)
                    idx_t = idxpool.tile([P, TPC, 8], u32, tag="idx")
                    for j in range(TPC):
                        t = cs * TPC + j
                        s_ps = spsum.tile([P, A], f32, tag="s")
                        for g in range(2):  # two 512-anchor halves
                            nc.tensor.matmul(
                                out=s_ps[:, g * 512 : (g + 1) * 512],
                                lhsT=cur_xh[
                                    32 * g : 32 * g + 5, j * P : (j + 1) * P
                                ],
                                rhs=kh_t[
                                    32 * g : 32 * g + 5, g * 512 : (g + 1) * 512
                                ],
                                start=True,
                                stop=True,
                                tile_position=(32 * g, 0),
                            )
                        m8 = m8_t[:, t * 8 : (t + 1) * 8]
                        nc.vector.max(m8, s_ps[:])
                        nc.vector.max_index(idx_t[:, j, :], m8, s_ps[:])
                    nc.sync.dma_start(
                        out=idx_d[:, cs * TPC * 8 : (cs + 1) * TPC * 8],
                        in_=idx_t[:],
                    )

                # --- MLP stage for chunk cm ----------------------------
                if cm >= 0:
                    vg_ts = vg_all[cm]
                    for half in range(2):
                        rlt_ps = ropsum.tile([4, 512], f16, tag="ro")
                        for q in range(4):
                            j = half * 4 + q
                            rl = rlpool.tile([P, 4], f16, tag="rl")
                            nc.gpsimd.tensor_tensor(
                                out=rl[:],
                                in0=x2_t[:, cm * TPC + j, :],
                                in1=vg_ts[j][:, D_OUT : D_OUT + 4],
                                op=mybir.AluOpType.subtract,
                            )
                            nc.tensor.transpose(
                                out=rlt_ps[:, q * P : (q + 1) * P],
                                in_=rl[:],
                                identity=id_t[:],
                            )
                        rlts = rltspool.tile([4, 512], f16, tag="rlts")
                        nc.scalar.activation(
                            out=rlts[:],
                            in_=rlt_ps[:],
                            func=mybir.ActivationFunctionType.Copy,
                        )

                        hts = []
                        for hc in range(HC):
                            h_ps = hpsum.tile([P, 512], f32, tag="h")
                            nc.tensor.matmul(
                                out=h_ps[:],
                                lhsT=w1h_t[:, hc * P : (hc + 1) * P],
                                rhs=rlts[:],
                                start=True,
                                stop=True,
                            )
                            ht = htpool.tile([P, 512], f16, tag="ht")
                            nc.scalar.activation(
                                out=ht[:],
                                in_=h_ps[:],
                                func=mybir.ActivationFunctionType.Gelu,
                                bias=b1p_t[:, hc : hc + 1],
                            )
                            hts.append(ht)

                        ob = None
                        for q in range(4):
                            j = half * 4 + q
                            t = cm * TPC + j
                            o_ps = ropsum.tile([P, D_OUT], f32, tag="ro")
                            for hc in range(HC):
                                nc.tensor.matmul(
                                    out=o_ps[:],
                                    lhsT=hts[hc][:, q * P : (q + 1) * P],
                                    rhs=w2p_t[:, hc * D_OUT : (hc + 1) * D_OUT],
                                    start=(hc == 0),
                                    stop=(hc == HC - 1),
                                )
                            qq = q % 2
                            if qq == 0:
                                ob = obpool.tile([P, 2, D_OUT], f32, tag="ob")
                            # out_psum + V row in one DVE op (reads PSUM)
                            nc.vector.tensor_tensor(
                                out=ob[:, qq, :],
                                in0=o_ps[:],
                                in1=vg_ts[j][:, 0:D_OUT],
                                op=mybir.AluOpType.add,
                            )
                            if qq == 1:
                                t0 = t - 1
                                nc.sync.dma_start(
                                    out=out_d[t0 * P : (t0 + 2) * P, :].rearrange(
                                        "(q p) o -> p q o", q=2
                                    ),
                                    in_=ob[:],
                                )

                # --- gather stage for chunk cs (GPS, after its subs) ---
                if cs < NCHUNK:
                    # NB: one offset per partition ([P,1]) per call into an
                    # offset-0 [P, W] dest tile - both multi-index offsets
                    # and non-zero dest offsets are mishandled by the real
                    # SWDGE (CoreSim accepts them but hardware does not).
                    vg_ts = []
                    for j in range(TPC):
                        vg_j = vgpool.tile([P, VT_W], f32, tag="vg")
                        nc.gpsimd.indirect_dma_start(
                            out=vg_j[:],
                            out_offset=None,
                            in_=vt_d[:],
                            in_offset=bass.IndirectOffsetOnAxis(
                                ap=idx_t[:, j, 0:1], axis=0
                            ),
                        )
                        vg_ts.append(vg_j)
                    vg_all[cs] = vg_ts

            nc.sync.dma_start(out=m8_d[:], in_=m8_t[:])

    nc.compile()
    names = ["xh", "kh", "x2", "w1h", "b1p", "w2p", "vt", "ident"]
    return nc, names


def _get_program():
    global _PROGRAM
    if _PROGRAM is None:
        _PROGRAM = _build_program()
    return _PROGRAM


def _host_pack(x, Key, init_mat, Value, w1, b1, w2, b2):
    """Build per-core input dicts (host-side layout packing)."""
    f = np.float32
    Key = np.asarray(Key, f)
    x = np.asarray(x, f)
    k2 = np.sum(Key * Key, axis=1)  # [A]

    # khat rows: [k0,k1,k2,1,|k|^2]; s = 2x.k - |x|^2 - |k|^2 = -d2
    kh = np.zeros((P, A), np.float16)
    kf = np.concatenate([Key, np.ones((A, 1), f), k2[:, None]], axis=1)  # [A,5]
    for g in range(2):
        kh[32 * g : 32 * g + 5, :] = kf.T.astype(np.float16)

    w1h = np.zeros((4, H), np.float16)
    w1h[:3, :] = (0.5 * np.asarray(w1, f)).astype(np.float16)
    b1p = np.asarray(b1, f).reshape(H // P, P).T.copy()  # [128, 8]
    w2p = (
        np.asarray(w2, f)
        .reshape(H // P, P, D_OUT)
        .transpose(1, 0, 2)
        .reshape(P, (H // P) * D_OUT)
        .astype(np.float16)
    )
    vt = np.zeros((A, VT_W), f)
    vt[:, :D_OUT] = np.asarray(init_mat, f) + np.asarray(Value, f) + np.asarray(b2, f)
    vt[:, D_OUT : D_OUT + 3] = 2.0 * Key
    ident = np.eye(P, dtype=np.float16)

    in_maps = []
    for c in range(N_CORES):
        xc = x[c]  # [N, 3]
        x2sq = np.sum(xc * xc, axis=1)  # [N]
        # xhat features [N, 5]: [2x, -|x|^2, -1]
        xf = np.concatenate(
            [2.0 * xc, -x2sq[:, None], -np.ones((N, 1), f)], axis=1
        ).astype(f)
        # packed lhsT [5, N]: tile t at cols t*128..; device replicates to
        # partition rows 0-4 and 32-36 via two DMAs.
        xh = (
            xf.reshape(NT, P, 5).transpose(2, 0, 1).reshape(5, N)
        ).astype(np.float16)

        x2q = np.zeros((N, 4), f)
        x2q[:, :3] = 2.0 * xc
        x2 = x2q.reshape(NT, P, 4).transpose(1, 0, 2).reshape(P, NT * 4).copy()

        in_maps.append(
            {
                "xh": xh,
                "kh": kh,
                "x2": x2,
                "w1h": w1h,
                "b1p": b1p,
                "w2p": w2p,
                "vt": vt,
                "ident": ident,
            }
        )
    return in_maps


def _erf(z):
    # Abramowitz-Stegun is not enough; use the exact erf from scipy if
    # present, else jax (available wherever the bass stack runs).
    try:
        from scipy.special import erf

        return erf(z)
    except ImportError:
        import jax

        with jax.default_device(jax.devices("cpu")[0]):
            return np.asarray(jax.scipy.special.erf(np.asarray(z, np.float32)))


def _refine(out, m8o, idxo, x, Key, init_mat, Value, w1, b1, w2, b2, tau=0.03):
    """Re-resolve tokens whose top-2 score gap is within tau (near-ties):
    recompute their argmin + output row in exact fp32 reference arithmetic."""
    f = np.float32
    Key = np.asarray(Key, f)
    V = np.asarray(init_mat, f) + np.asarray(Value, f)
    k2 = np.sum(Key * Key, axis=1)
    n_fixed = 0
    for c in range(out.shape[0]):
        m8 = m8o[c]  # [128, NT*8]
        m0 = m8[:, 0::8]  # [128, NT]
        m1 = m8[:, 1::8]
        gap = m0 - m1  # s-space gap == d2 second - d2 min
        dev_idx = idxo[c][:, 0::8].astype(np.int64)  # [128, NT]
        scale = 1.0 + np.abs(m0)
        flag = gap < tau * scale  # [128, NT]
        ps, ts = np.nonzero(flag)
        if ps.size == 0:
            continue
        toks = ts * P + ps
        xc = np.asarray(x[c], f)[toks]  # [F, 3]
        d2 = -2.0 * (xc @ Key.T) + k2[None, :]  # reference formula, fp32
        amin = np.argmin(d2, axis=1)
        mism = amin != dev_idx[ps, ts]
        if not np.any(mism):
            continue
        toks = toks[mism]
        amin = amin[mism]
        xe = np.asarray(x[c], f)[toks]
        rl = xe - Key[amin]
        pre = (rl @ np.asarray(w1, f) + np.asarray(b1, f)).astype(f)
        h = (0.5 * pre * (1.0 + _erf(pre / np.sqrt(f(2.0))))).astype(f)
        row = (h @ np.asarray(w2, f) + np.asarray(b2, f) + V[amin]).astype(f)
        out[c, toks, :] = row
        n_fixed += toks.size
    return n_fixed


def kernel(**inputs):
    from concourse.bass_utils import run_bass_kernel_spmd

    nc, names = _get_program()
    in_maps = _host_pack(**inputs)
    res = run_bass_kernel_spmd(nc, in_maps, core_ids=list(range(N_CORES)))

    out = np.zeros((B, N, D_OUT), np.float32)
    m8o = np.zeros((B, P, NT * 8), np.float32)
    idxo = np.zeros((B, P, NT * 8), np.uint32)
    for c in range(N_CORES):
        r = res.results[c]
        out[c] = r["outp"]
        m8o[c] = r["m8o"]
        idxo[c] = r["idxo"]

    _refine(out, m8o, idxo, **inputs)
    return out


if __name__ == "__main__":
    # smoke: build only
    _get_program()
    print("program built")


# revision 4
# speedup vs baseline: 1.3985x; 1.0226x over previous
"""FAISS-anchor kernel layer on 8 Trainium2 NeuronCores (Bass/Tile).

Problem (per full input):
    x [8,8192,3], Key [1024,3], init_mat/Value [1024,256],
    w1 [3,1024], b1 [1024], w2 [1024,256], b2 [256]
    idx = argmin_a ||x - Key_a||^2           (exact 1-NN, first-tie)
    out = gelu((x - Key[idx]) @ w1 + b1) @ w2 + b2 + (init_mat + Value)[idx]

Sharding: pure data-parallel - core c takes batch element c (8192 tokens).
All tables (Key-derived features, V-table, MLP weights) are replicated.

v2 design (vs the f32r baseline):
  * fp16 PE operands everywhere (scores, w1, rl^T, gelu out, w2): PE streams
    16-bit at ~2.4x the f32r rate and fp16's 10-bit mantissa keeps the same
    near-tie refine threshold (tau=0.03) as f32r.
  * software pipeline: round r runs chunk r's score/argmax/gather stage
    against chunk r-1's MLP stage so PE never waits on the DVE->GPS chain.
  * engine re-balance: GPSIMD only does the indirect gather + tiny rl
    subtract; the V-add runs on DVE straight out of PSUM (kills the
    ACT copy + GPS add of the baseline and their semaphores).
  * PSUM: scores 2x2 banks, h 2x1, shared {rl^T, out} tag ring 2x1 = 8.

Device pipeline per 128-token tile:
    PE:   s = -||x-k||^2 for all 1024 anchors (two row-group-packed fp16
          matmuls with [2x, -|x|^2, -1] x [k, 1, |k|^2] feature folding)
    DVE:  max8 -> top-8 of s; max_index -> anchor index
    GPS:  indirect gather of fused table row [V+init+b2 | 2*Key] per token;
          rl2 = 2x - 2*Key[idx] (fp16 out)
    PE:   transpose rl2 -> [4, tok] (fp16); h^T = (0.5*w1)^T @ rl2^T (K=4)
    ACT:  rl^T PSUM->SBUF fp16 copy; gelu(h^T + b1) -> fp16
    PE:   out = h @ w2 (h^T tiles stationary, K=8x128, fp16)
    DVE:  ob = out_psum + gathered V row; paired-store DMA out.

Host: packs layouts, runs 8 cores via run_bass_kernel_spmd, re-assembles,
and re-resolves near-tie tokens (top-2 gap below tau) with exact fp32
reference arithmetic so fp16 matmul rounding cannot flip the argmin.
"""

import numpy as np

B, N, A, D_IN, D_OUT = 8, 8192, 1024, 3, 256
H = 4 * D_OUT
P = 128
NT = N // P            # 64 token tiles per core
TPC = 8                # tiles per chunk
NCHUNK = NT // TPC     # 8 chunks
VT_W = 264             # gather-table row width (256 V + 3 key + pad), 1056B
N_CORES = 8

_PROGRAM = None  # (nc, input_names)


def _build_program():
    import concourse.bass as bass
    import concourse.mybir as mybir
    import concourse.tile as tile
    from concourse import bacc

    f32 = mybir.dt.float32
    f16 = mybir.dt.float16
    u32 = mybir.dt.uint32

    # Bacc (not raw Bass): its compile() splits multi-sem waits and moves
    # matmul waits onto ldweights - TRN2 allows at most 1 wait per instr.
    nc = bacc.Bacc("TRN2", target_bir_lowering=False, debug=False)

    # DRAM I/O
    xh_d = nc.dram_tensor("xh", [5, N], f16, kind="ExternalInput").ap()
    kh_d = nc.dram_tensor("kh", [P, A], f16, kind="ExternalInput").ap()
    x2_d = nc.dram_tensor("x2", [P, NT * 4], f32, kind="ExternalInput").ap()
    w1h_d = nc.dram_tensor("w1h", [4, H], f16, kind="ExternalInput").ap()
    b1p_d = nc.dram_tensor("b1p", [P, H // P], f32, kind="ExternalInput").ap()
    w2p_d = nc.dram_tensor("w2p", [P, (H // P) * D_OUT], f16, kind="ExternalInput").ap()
    vt_d = nc.dram_tensor("vt", [A, VT_W], f32, kind="ExternalInput").ap()
    id_d = nc.dram_tensor("ident", [P, P], f16, kind="ExternalInput").ap()

    out_d = nc.dram_tensor("outp", [N, D_OUT], f32, kind="ExternalOutput").ap()
    m8_d = nc.dram_tensor("m8o", [P, NT * 8], f32, kind="ExternalOutput").ap()
    idx_d = nc.dram_tensor("idxo", [P, NT * 8], u32, kind="ExternalOutput").ap()

    HC = H // P  # 8 H-chunks

    with tile.TileContext(nc) as tc:
        with (
            tc.tile_pool(name="const", bufs=1) as cpool,
            tc.tile_pool(name="xh", bufs=2) as xhpool,
            tc.tile_pool(name="vg", bufs=16) as vgpool,
            tc.tile_pool(name="rl", bufs=4) as rlpool,
            tc.tile_pool(name="rlts", bufs=2) as rltspool,
            tc.tile_pool(name="ht", bufs=16) as htpool,
            tc.tile_pool(name="m8", bufs=1) as m8pool,
            tc.tile_pool(name="idx", bufs=2) as idxpool,
            tc.tile_pool(name="ob", bufs=3) as obpool,
            tc.tile_pool(name="spsum", bufs=2, space="PSUM") as spsum,
            tc.tile_pool(name="hpsum", bufs=2, space="PSUM") as hpsum,
            tc.tile_pool(name="ropsum", bufs=2, space="PSUM") as ropsum,
        ):
            # Resident constants
            kh_t = cpool.tile([P, A], f16)
            nc.sync.dma_start(out=kh_t[:], in_=kh_d[:])
            x2_t = cpool.tile([P, NT, 4], f32)
            nc.sync.dma_start(out=x2_t[:], in_=x2_d[:])
            w1h_t = cpool.tile([4, H], f16)
            nc.sync.dma_start(out=w1h_t[:], in_=w1h_d[:])
            b1p_t = cpool.tile([P, HC], f32)
            nc.sync.dma_start(out=b1p_t[:], in_=b1p_d[:])
            w2p_t = cpool.tile([P, HC * D_OUT], f16)
            nc.sync.dma_start(out=w2p_t[:], in_=w2p_d[:])
            id_t = cpool.tile([P, P], f16)
            nc.sync.dma_start(out=id_t[:], in_=id_d[:])
            m8_t = m8pool.tile([P, NT * 8], f32)

            # xh tiles: features at partition rows 0-4 plus a replica at
            # rows 32-36 for 2-way PE row-group packing.
            def load_xh(c):
                xh_t = xhpool.tile([37, TPC * P], f16, tag="xh")
                sl = xh_d[:, c * TPC * P : (c + 1) * TPC * P]
                nc.sync.dma_start(out=xh_t[0:5, :], in_=sl)
                nc.sync.dma_start(out=xh_t[32:37, :], in_=sl)
                return xh_t

            xh_t = load_xh(0)
            vg_all = [None] * NCHUNK  # per-chunk gathered rows, live 1 round

            def emit_score_pair(cs, j0, cur_xh, idx_t):
                for j in (j0, j0 + 1):
                    t = cs * TPC + j
                    s_ps = spsum.tile([P, A], f32, tag="s")
                    for g in range(2):  # two 512-anchor halves
                        nc.tensor.matmul(
                            out=s_ps[:, g * 512 : (g + 1) * 512],
                            lhsT=cur_xh[
                                32 * g : 32 * g + 5, j * P : (j + 1) * P
                            ],
                            rhs=kh_t[
                                32 * g : 32 * g + 5, g * 512 : (g + 1) * 512
                            ],
                            start=True,
                            stop=True,
                            tile_position=(32 * g, 0),
                        )
                    m8 = m8_t[:, t * 8 : (t + 1) * 8]
                    nc.vector.max(m8, s_ps[:])
                    nc.vector.max_index(idx_t[:, j, :], m8, s_ps[:])

            def emit_prep(c, half):
                # rl = 2x - 2*Key[idx] (GPS), transpose to [4, tok] (PE),
                # PSUM->SBUF fp16 copy (ACT). Runs one stage ahead of main.
                vg_ts = vg_all[c]
                rlt_ps = ropsum.tile([4, 512], f16, tag="ro")
                for q in range(4):
                    j = half * 4 + q
                    rl = rlpool.tile([P, 4], f16, tag="rl")
                    nc.gpsimd.tensor_tensor(
                        out=rl[:],
                        in0=x2_t[:, c * TPC + j, :],
                        in1=vg_ts[j][:, D_OUT : D_OUT + 4],
                        op=mybir.AluOpType.subtract,
                    )
                    nc.tensor.transpose(
                        out=rlt_ps[:, q * P : (q + 1) * P],
                        in_=rl[:],
                        identity=id_t[:],
                    )
                rlts = rltspool.tile([4, 512], f16, tag="rlts")
                nc.scalar.activation(
                    out=rlts[:],
                    in_=rlt_ps[:],
                    func=mybir.ActivationFunctionType.Copy,
                )
                return rlts

            def emit_main_h(rlts):
                hts = []
                for hc in range(HC):
                    h_ps = hpsum.tile([P, 512], f32, tag="h")
                    nc.tensor.matmul(
                        out=h_ps[:],
                        lhsT=w1h_t[:, hc * P : (hc + 1) * P],
                        rhs=rlts[:],
                        start=True,
                        stop=True,
                    )
                    ht = htpool.tile([P, 512], f16, tag="ht")
                    nc.scalar.activation(
                        out=ht[:],
                        in_=h_ps[:],
                        func=mybir.ActivationFunctionType.Gelu,
                        bias=b1p_t[:, hc : hc + 1],
                    )
                    hts.append(ht)
                return hts

            def emit_main_o(c, half, hts, q0):
                vg_ts = vg_all[c]
                ob = obpool.tile([P, 2, D_OUT], f32, tag="ob")
                for q in (q0, q0 + 1):
                    j = half * 4 + q
                    t = c * TPC + j
                    o_ps = ropsum.tile([P, D_OUT], f32, tag="ro")
                    for hc in range(HC):
                        nc.tensor.matmul(
                            out=o_ps[:],
                            lhsT=hts[hc][:, q * P : (q + 1) * P],
                            rhs=w2p_t[:, hc * D_OUT : (hc + 1) * D_OUT],
                            start=(hc == 0),
                            stop=(hc == HC - 1),
                        )
                    # out_psum + V row in one DVE op (reads PSUM)
                    nc.vector.tensor_tensor(
                        out=ob[:, q % 2, :],
                        in0=o_ps[:],
                        in1=vg_ts[j][:, 0:D_OUT],
                        op=mybir.AluOpType.add,
                    )
                t0 = c * TPC + half * 4 + q0
                nc.sync.dma_start(
                    out=out_d[t0 * P : (t0 + 2) * P, :].rearrange(
                        "(q p) o -> p q o", q=2
                    ),
                    in_=ob[:],
                )

            rlts_h0 = None  # prep(cm, 0) result, emitted in round r-1
            for r in range(NCHUNK + 1):
                cs, cm = r, r - 1
                have_s = cs < NCHUNK
                have_m = cm >= 0

                if have_s:
                    cur_xh = xh_t
                    if cs + 1 < NCHUNK:
                        xh_t = load_xh(cs + 1)
                    idx_t = idxpool.tile([P, TPC, 8], u32, tag="idx")
                    emit_score_pair(cs, 0, cur_xh, idx_t)
                if have_m:
                    hts0 = emit_main_h(rlts_h0)
                if have_s:
                    emit_score_pair(cs, 2, cur_xh, idx_t)
                if have_m:
                    emit_main_o(cm, 0, hts0, 0)
                if have_s:
                    emit_score_pair(cs, 4, cur_xh, idx_t)
                if have_m:
                    emit_main_o(cm, 0, hts0, 2)
                if have_s:
                    emit_score_pair(cs, 6, cur_xh, idx_t)
                    nc.sync.dma_start(
                        out=idx_d[:, cs * TPC * 8 : (cs + 1) * TPC * 8],
                        in_=idx_t[:],
                    )
                if have_m:
                    rlts1 = emit_prep(cm, 1)
                    hts1 = emit_main_h(rlts1)
                    emit_main_o(cm, 1, hts1, 0)
                    emit_main_o(cm, 1, hts1, 2)

                # --- gather stage for chunk cs (GPS, after cm's subs) --
                if have_s:
                    # NB: one offset per partition ([P,1]) per call into an
                    # offset-0 [P, W] dest tile - both multi-index offsets
                    # and non-zero dest offsets are mishandled by the real
                    # SWDGE (CoreSim accepts them but hardware does not).
                    vg_ts = []
                    for j in range(TPC):
                        vg_j = vgpool.tile([P, VT_W], f32, tag="vg")
                        nc.gpsimd.indirect_dma_start(
                            out=vg_j[:],
                            out_offset=None,
                            in_=vt_d[:],
                            in_offset=bass.IndirectOffsetOnAxis(
                                ap=idx_t[:, j, 0:1], axis=0
                            ),
                        )
                        vg_ts.append(vg_j)
                    vg_all[cs] = vg_ts
                    # prep for next round's first half, right behind the
                    # gathers it depends on
                    rlts_h0 = emit_prep(cs, 0)

            nc.sync.dma_start(out=m8_d[:], in_=m8_t[:])

    nc.compile()
    names = ["xh", "kh", "x2", "w1h", "b1p", "w2p", "vt", "ident"]
    return nc, names


def _get_program():
    global _PROGRAM
    if _PROGRAM is None:
        _PROGRAM = _build_program()
    return _PROGRAM


def _host_pack(x, Key, init_mat, Value, w1, b1, w2, b2):
    """Build per-core input dicts (host-side layout packing)."""
    f = np.float32
    Key = np.asarray(Key, f)
    x = np.asarray(x, f)
    k2 = np.sum(Key * Key, axis=1)  # [A]

    # khat rows: [k0,k1,k2,1,|k|^2]; s = 2x.k - |x|^2 - |k|^2 = -d2
    kh = np.zeros((P, A), np.float16)
    kf = np.concatenate([Key, np.ones((A, 1), f), k2[:, None]], axis=1)  # [A,5]
    for g in range(2):
        kh[32 * g : 32 * g + 5, :] = kf.T.astype(np.float16)

    w1h = np.zeros((4, H), np.float16)
    w1h[:3, :] = (0.5 * np.asarray(w1, f)).astype(np.float16)
    b1p = np.asarray(b1, f).reshape(H // P, P).T.copy()  # [128, 8]
    w2p = (
        np.asarray(w2, f)
        .reshape(H // P, P, D_OUT)
        .transpose(1, 0, 2)
        .reshape(P, (H // P) * D_OUT)
        .astype(np.float16)
    )
    vt = np.zeros((A, VT_W), f)
    vt[:, :D_OUT] = np.asarray(init_mat, f) + np.asarray(Value, f) + np.asarray(b2, f)
    vt[:, D_OUT : D_OUT + 3] = 2.0 * Key
    ident = np.eye(P, dtype=np.float16)

    in_maps = []
    for c in range(N_CORES):
        xc = x[c]  # [N, 3]
        x2sq = np.sum(xc * xc, axis=1)  # [N]
        # xhat features [N, 5]: [2x, -|x|^2, -1]
        xf = np.concatenate(
            [2.0 * xc, -x2sq[:, None], -np.ones((N, 1), f)], axis=1
        ).astype(f)
        # packed lhsT [5, N]: tile t at cols t*128..; device replicates to
        # partition rows 0-4 and 32-36 via two DMAs.
        xh = (
            xf.reshape(NT, P, 5).transpose(2, 0, 1).reshape(5, N)
        ).astype(np.float16)

        x2q = np.zeros((N, 4), f)
        x2q[:, :3] = 2.0 * xc
        x2 = x2q.reshape(NT, P, 4).transpose(1, 0, 2).reshape(P, NT * 4).copy()

        in_maps.append(
            {
                "xh": xh,
                "kh": kh,
                "x2": x2,
                "w1h": w1h,
                "b1p": b1p,
                "w2p": w2p,
                "vt": vt,
                "ident": ident,
            }
        )
    return in_maps


def _erf(z):
    # Abramowitz-Stegun is not enough; use the exact erf from scipy if
    # present, else jax (available wherever the bass stack runs).
    try:
        from scipy.special import erf

        return erf(z)
    except ImportError:
        import jax

        with jax.default_device(jax.devices("cpu")[0]):
            return np.asarray(jax.scipy.special.erf(np.asarray(z, np.float32)))


def _refine(out, m8o, idxo, x, Key, init_mat, Value, w1, b1, w2, b2, tau=0.03):
    """Re-resolve tokens whose top-2 score gap is within tau (near-ties):
    recompute their argmin + output row in exact fp32 reference arithmetic."""
    f = np.float32
    Key = np.asarray(Key, f)
    V = np.asarray(init_mat, f) + np.asarray(Value, f)
    k2 = np.sum(Key * Key, axis=1)
    n_fixed = 0
    for c in range(out.shape[0]):
        m8 = m8o[c]  # [128, NT*8]
        m0 = m8[:, 0::8]  # [128, NT]
        m1 = m8[:, 1::8]
        gap = m0 - m1  # s-space gap == d2 second - d2 min
        dev_idx = idxo[c][:, 0::8].astype(np.int64)  # [128, NT]
        scale = 1.0 + np.abs(m0)
        flag = gap < tau * scale  # [128, NT]
        ps, ts = np.nonzero(flag)
        if ps.size == 0:
            continue
        toks = ts * P + ps
        xc = np.asarray(x[c], f)[toks]  # [F, 3]
        d2 = -2.0 * (xc @ Key.T) + k2[None, :]  # reference formula, fp32
        amin = np.argmin(d2, axis=1)
        mism = amin != dev_idx[ps, ts]
        if not np.any(mism):
            continue
        toks = toks[mism]
        amin = amin[mism]
        xe = np.asarray(x[c], f)[toks]
        rl = xe - Key[amin]
        pre = (rl @ np.asarray(w1, f) + np.asarray(b1, f)).astype(f)
        h = (0.5 * pre * (1.0 + _erf(pre / np.sqrt(f(2.0))))).astype(f)
        row = (h @ np.asarray(w2, f) + np.asarray(b2, f) + V[amin]).astype(f)
        out[c, toks, :] = row
        n_fixed += toks.size
    return n_fixed


def kernel(**inputs):
    from concourse.bass_utils import run_bass_kernel_spmd

    nc, names = _get_program()
    in_maps = _host_pack(**inputs)
    res = run_bass_kernel_spmd(nc, in_maps, core_ids=list(range(N_CORES)))

    out = np.zeros((B, N, D_OUT), np.float32)
    m8o = np.zeros((B, P, NT * 8), np.float32)
    idxo = np.zeros((B, P, NT * 8), np.uint32)
    for c in range(N_CORES):
        r = res.results[c]
        out[c] = r["outp"]
        m8o[c] = r["m8o"]
        idxo[c] = r["idxo"]

    _refine(out, m8o, idxo, **inputs)
    return out


if __name__ == "__main__":
    # smoke: build only
    _get_program()
    print("program built")


# revision 6
# speedup vs baseline: 1.6016x; 1.1452x over previous
"""FAISS-anchor kernel layer on 8 Trainium2 NeuronCores (Bass/Tile).

Problem (per full input):
    x [8,8192,3], Key [1024,3], init_mat/Value [1024,256],
    w1 [3,1024], b1 [1024], w2 [1024,256], b2 [256]
    idx = argmin_a ||x - Key_a||^2           (exact 1-NN, first-tie)
    out = gelu((x - Key[idx]) @ w1 + b1) @ w2 + b2 + (init_mat + Value)[idx]

Sharding: pure data-parallel - core c takes batch element c (8192 tokens).
All tables (Key-derived features, V-table, MLP weights) are replicated.

v2 design (vs the f32r baseline):
  * fp16 PE operands everywhere (scores, w1, rl^T, gelu out, w2): PE streams
    16-bit at ~2.4x the f32r rate and fp16's 10-bit mantissa keeps the same
    near-tie refine threshold (tau=0.03) as f32r.
  * software pipeline: round r runs chunk r's score/argmax/gather stage
    against chunk r-1's MLP stage so PE never waits on the DVE->GPS chain.
  * engine re-balance: GPSIMD only does the indirect gather + tiny rl
    subtract; the V-add runs on DVE straight out of PSUM (kills the
    ACT copy + GPS add of the baseline and their semaphores).
  * PSUM: scores 2x2 banks, h 2x1, shared {rl^T, out} tag ring 2x1 = 8.

Device pipeline per 128-token tile:
    PE:   s = -||x-k||^2 for all 1024 anchors (two row-group-packed fp16
          matmuls with [2x, -|x|^2, -1] x [k, 1, |k|^2] feature folding)
    DVE:  max8 -> top-8 of s; max_index -> anchor index
    GPS:  indirect gather of fused table row [V+init+b2 | 2*Key] per token;
          rl2 = 2x - 2*Key[idx] (fp16 out)
    PE:   transpose rl2 -> [4, tok] (fp16); h^T = (0.5*w1)^T @ rl2^T (K=4)
    ACT:  rl^T PSUM->SBUF fp16 copy; gelu(h^T + b1) -> fp16
    PE:   out = h @ w2 (h^T tiles stationary, K=8x128, fp16)
    DVE:  ob = out_psum + gathered V row; paired-store DMA out.

Host: packs layouts, runs 8 cores via run_bass_kernel_spmd, re-assembles,
and re-resolves near-tie tokens (top-2 gap below tau) with exact fp32
reference arithmetic so fp16 matmul rounding cannot flip the argmin.
"""

import numpy as np

B, N, A, D_IN, D_OUT = 8, 8192, 1024, 3, 256
H = 4 * D_OUT
P = 128
NT = N // P            # 64 token tiles per core
TPC = 8                # tiles per chunk
NCHUNK = NT // TPC     # 8 chunks
VT_W = 264             # gather-table row width (256 V + 3 key + pad), 1056B
N_CORES = 8

_PROGRAM = None  # (nc, input_names)


def _build_program():
    import concourse.bass as bass
    import concourse.mybir as mybir
    import concourse.tile as tile
    from concourse import bacc

    f32 = mybir.dt.float32
    f16 = mybir.dt.float16
    u32 = mybir.dt.uint32

    # Bacc (not raw Bass): its compile() splits multi-sem waits and moves
    # matmul waits onto ldweights - TRN2 allows at most 1 wait per instr.
    nc = bacc.Bacc("TRN2", target_bir_lowering=False, debug=False)

    # DRAM I/O
    xh_d = nc.dram_tensor("xh", [5, N], f16, kind="ExternalInput").ap()
    kh_d = nc.dram_tensor("kh", [P, A], f16, kind="ExternalInput").ap()
    x2_d = nc.dram_tensor("x2", [P, NT * 4], f32, kind="ExternalInput").ap()
    w1h_d = nc.dram_tensor("w1h", [4, H], f16, kind="ExternalInput").ap()
    b1p_d = nc.dram_tensor("b1p", [P, H // P], f32, kind="ExternalInput").ap()
    w2p_d = nc.dram_tensor("w2p", [P, (H // P) * D_OUT], f16, kind="ExternalInput").ap()
    vt_d = nc.dram_tensor("vt", [A, VT_W], f16, kind="ExternalInput").ap()
    id_d = nc.dram_tensor("ident", [P, P], f16, kind="ExternalInput").ap()

    out_d = nc.dram_tensor("outp", [N, D_OUT], f32, kind="ExternalOutput").ap()
    m8_d = nc.dram_tensor("m8o", [P, NT * 8], f32, kind="ExternalOutput").ap()
    idx_d = nc.dram_tensor("idxo", [P, NT * 8], u32, kind="ExternalOutput").ap()

    HC = H // P  # 8 H-chunks

    with tile.TileContext(nc) as tc:
        with (
            tc.tile_pool(name="const", bufs=1) as cpool,
            tc.tile_pool(name="xh", bufs=2) as xhpool,
            tc.tile_pool(name="vg", bufs=16) as vgpool,
            tc.tile_pool(name="rl", bufs=4) as rlpool,
            tc.tile_pool(name="rlts", bufs=2) as rltspool,
            tc.tile_pool(name="ht", bufs=16) as htpool,
            tc.tile_pool(name="m8", bufs=1) as m8pool,
            tc.tile_pool(name="idx", bufs=2) as idxpool,
            tc.tile_pool(name="ob", bufs=3) as obpool,
            tc.tile_pool(name="spsum", bufs=2, space="PSUM") as spsum,
            tc.tile_pool(name="hpsum", bufs=2, space="PSUM") as hpsum,
            tc.tile_pool(name="ropsum", bufs=2, space="PSUM") as ropsum,
        ):
            # Resident constants
            kh_t = cpool.tile([P, A], f16)
            nc.sync.dma_start(out=kh_t[:], in_=kh_d[:])
            x2_t = cpool.tile([P, NT, 4], f32)
            nc.sync.dma_start(out=x2_t[:], in_=x2_d[:])
            w1h_t = cpool.tile([4, H], f16)
            nc.sync.dma_start(out=w1h_t[:], in_=w1h_d[:])
            b1p_t = cpool.tile([P, HC], f32)
            nc.sync.dma_start(out=b1p_t[:], in_=b1p_d[:])
            w2p_t = cpool.tile([P, HC * D_OUT], f16)
            nc.sync.dma_start(out=w2p_t[:], in_=w2p_d[:])
            id_t = cpool.tile([P, P], f16)
            nc.sync.dma_start(out=id_t[:], in_=id_d[:])
            m8_t = m8pool.tile([P, NT * 8], f32)

            # xh tiles: features at partition rows 0-4 plus a replica at
            # rows 32-36 for 2-way PE row-group packing.
            def load_xh(c):
                xh_t = xhpool.tile([37, TPC * P], f16, tag="xh")
                sl = xh_d[:, c * TPC * P : (c + 1) * TPC * P]
                nc.sync.dma_start(out=xh_t[0:5, :], in_=sl)
                nc.sync.dma_start(out=xh_t[32:37, :], in_=sl)
                return xh_t

            xh_t = load_xh(0)
            vg_all = [None] * NCHUNK  # per-chunk gathered rows, live 1 round

            def emit_score_pair(cs, j0, cur_xh, idx_t):
                for j in (j0, j0 + 1):
                    t = cs * TPC + j
                    s_ps = spsum.tile([P, A], f32, tag="s")
                    for g in range(2):  # two 512-anchor halves
                        nc.tensor.matmul(
                            out=s_ps[:, g * 512 : (g + 1) * 512],
                            lhsT=cur_xh[
                                32 * g : 32 * g + 5, j * P : (j + 1) * P
                            ],
                            rhs=kh_t[
                                32 * g : 32 * g + 5, g * 512 : (g + 1) * 512
                            ],
                            start=True,
                            stop=True,
                            tile_position=(32 * g, 0),
                        )
                    m8 = m8_t[:, t * 8 : (t + 1) * 8]
                    nc.vector.max(m8, s_ps[:])
                    nc.vector.max_index(idx_t[:, j, :], m8, s_ps[:])

            def emit_prep(c, half):
                # rl = 2x - 2*Key[idx] (GPS), transpose to [4, tok] (PE),
                # PSUM->SBUF fp16 copy (ACT). Runs one stage ahead of main.
                vg_ts = vg_all[c]
                rlt_ps = ropsum.tile([4, 512], f16, tag="ro")
                for q in range(4):
                    j = half * 4 + q
                    rl = rlpool.tile([P, 4], f16, tag="rl")
                    nc.gpsimd.tensor_tensor(
                        out=rl[:],
                        in0=x2_t[:, c * TPC + j, :],
                        in1=vg_ts[j][:, D_OUT : D_OUT + 4],
                        op=mybir.AluOpType.subtract,
                    )
                    nc.tensor.transpose(
                        out=rlt_ps[:, q * P : (q + 1) * P],
                        in_=rl[:],
                        identity=id_t[:],
                    )
                rlts = rltspool.tile([4, 512], f16, tag="rlts")
                nc.scalar.activation(
                    out=rlts[:],
                    in_=rlt_ps[:],
                    func=mybir.ActivationFunctionType.Copy,
                )
                return rlts

            def emit_main_h(rlts):
                hts = []
                for hc in range(HC):
                    h_ps = hpsum.tile([P, 512], f32, tag="h")
                    nc.tensor.matmul(
                        out=h_ps[:],
                        lhsT=w1h_t[:, hc * P : (hc + 1) * P],
                        rhs=rlts[:],
                        start=True,
                        stop=True,
                    )
                    ht = htpool.tile([P, 512], f16, tag="ht")
                    nc.scalar.activation(
                        out=ht[:],
                        in_=h_ps[:],
                        func=mybir.ActivationFunctionType.Gelu,
                        bias=b1p_t[:, hc : hc + 1],
                    )
                    hts.append(ht)
                return hts

            def emit_main_o(c, half, hts, q0):
                vg_ts = vg_all[c]
                ob = obpool.tile([P, 2, D_OUT], f32, tag="ob")
                for q in (q0, q0 + 1):
                    j = half * 4 + q
                    t = c * TPC + j
                    o_ps = ropsum.tile([P, D_OUT], f32, tag="ro")
                    for hc in range(HC):
                        nc.tensor.matmul(
                            out=o_ps[:],
                            lhsT=hts[hc][:, q * P : (q + 1) * P],
                            rhs=w2p_t[:, hc * D_OUT : (hc + 1) * D_OUT],
                            start=(hc == 0),
                            stop=(hc == HC - 1),
                        )
                    # out_psum + V row in one DVE op (reads PSUM)
                    nc.vector.tensor_tensor(
                        out=ob[:, q % 2, :],
                        in0=o_ps[:],
                        in1=vg_ts[j][:, 0:D_OUT],
                        op=mybir.AluOpType.add,
                    )
                t0 = c * TPC + half * 4 + q0
                nc.sync.dma_start(
                    out=out_d[t0 * P : (t0 + 2) * P, :].rearrange(
                        "(q p) o -> p q o", q=2
                    ),
                    in_=ob[:],
                )

            rlts_h0 = None  # prep(cm, 0) result, emitted in round r-1
            for r in range(NCHUNK + 1):
                cs, cm = r, r - 1
                have_s = cs < NCHUNK
                have_m = cm >= 0

                if have_s:
                    cur_xh = xh_t
                    if cs + 1 < NCHUNK:
                        xh_t = load_xh(cs + 1)
                    idx_t = idxpool.tile([P, TPC, 8], u32, tag="idx")
                    emit_score_pair(cs, 0, cur_xh, idx_t)
                if have_m:
                    hts0 = emit_main_h(rlts_h0)
                if have_s:
                    emit_score_pair(cs, 2, cur_xh, idx_t)
                if have_m:
                    emit_main_o(cm, 0, hts0, 0)
                    # half1 prep here so its ACT copy hides under the
                    # out-matmuls of half0
                    rlts1 = emit_prep(cm, 1)
                if have_s:
                    emit_score_pair(cs, 4, cur_xh, idx_t)
                if have_m:
                    emit_main_o(cm, 0, hts0, 2)
                if have_s:
                    emit_score_pair(cs, 6, cur_xh, idx_t)
                    nc.sync.dma_start(
                        out=idx_d[:, cs * TPC * 8 : (cs + 1) * TPC * 8],
                        in_=idx_t[:],
                    )
                if have_m:
                    hts1 = emit_main_h(rlts1)
                    emit_main_o(cm, 1, hts1, 0)
                    emit_main_o(cm, 1, hts1, 2)

                # --- gather stage for chunk cs (GPS, after cm's subs) --
                if have_s:
                    # NB: one offset per partition ([P,1]) per call into an
                    # offset-0 [P, W] dest tile - both multi-index offsets
                    # and non-zero dest offsets are mishandled by the real
                    # SWDGE (CoreSim accepts them but hardware does not).
                    vg_ts = []
                    for j in range(TPC):
                        vg_j = vgpool.tile([P, VT_W], f16, tag="vg")
                        nc.gpsimd.indirect_dma_start(
                            out=vg_j[:],
                            out_offset=None,
                            in_=vt_d[:],
                            in_offset=bass.IndirectOffsetOnAxis(
                                ap=idx_t[:, j, 0:1], axis=0
                            ),
                        )
                        vg_ts.append(vg_j)
                    vg_all[cs] = vg_ts
                    # prep for next round's first half, right behind the
                    # gathers it depends on
                    rlts_h0 = emit_prep(cs, 0)

            nc.sync.dma_start(out=m8_d[:], in_=m8_t[:])

    nc.compile()
    names = ["xh", "kh", "x2", "w1h", "b1p", "w2p", "vt", "ident"]
    return nc, names


def _get_program():
    global _PROGRAM
    if _PROGRAM is None:
        _PROGRAM = _build_program()
    return _PROGRAM


def _host_pack(x, Key, init_mat, Value, w1, b1, w2, b2):
    """Build per-core input dicts (host-side layout packing)."""
    f = np.float32
    Key = np.asarray(Key, f)
    x = np.asarray(x, f)
    k2 = np.sum(Key * Key, axis=1)  # [A]

    # khat rows: [k0,k1,k2,1,|k|^2]; s = 2x.k - |x|^2 - |k|^2 = -d2
    kh = np.zeros((P, A), np.float16)
    kf = np.concatenate([Key, np.ones((A, 1), f), k2[:, None]], axis=1)  # [A,5]
    for g in range(2):
        kh[32 * g : 32 * g + 5, :] = kf.T.astype(np.float16)

    w1h = np.zeros((4, H), np.float16)
    w1h[:3, :] = (0.5 * np.asarray(w1, f)).astype(np.float16)
    b1p = np.asarray(b1, f).reshape(H // P, P).T.copy()  # [128, 8]
    w2p = (
        np.asarray(w2, f)
        .reshape(H // P, P, D_OUT)
        .transpose(1, 0, 2)
        .reshape(P, (H // P) * D_OUT)
        .astype(np.float16)
    )
    vt = np.zeros((A, VT_W), np.float16)
    vt[:, :D_OUT] = np.asarray(init_mat, f) + np.asarray(Value, f) + np.asarray(b2, f)
    vt[:, D_OUT : D_OUT + 3] = 2.0 * Key
    ident = np.eye(P, dtype=np.float16)

    in_maps = []
    for c in range(N_CORES):
        xc = x[c]  # [N, 3]
        x2sq = np.sum(xc * xc, axis=1)  # [N]
        # xhat features [N, 5]: [2x, -|x|^2, -1]
        xf = np.concatenate(
            [2.0 * xc, -x2sq[:, None], -np.ones((N, 1), f)], axis=1
        ).astype(f)
        # packed lhsT [5, N]: tile t at cols t*128..; device replicates to
        # partition rows 0-4 and 32-36 via two DMAs.
        xh = (
            xf.reshape(NT, P, 5).transpose(2, 0, 1).reshape(5, N)
        ).astype(np.float16)

        x2q = np.zeros((N, 4), f)
        x2q[:, :3] = 2.0 * xc
        x2 = x2q.reshape(NT, P, 4).transpose(1, 0, 2).reshape(P, NT * 4).copy()

        in_maps.append(
            {
                "xh": xh,
                "kh": kh,
                "x2": x2,
                "w1h": w1h,
                "b1p": b1p,
                "w2p": w2p,
                "vt": vt,
                "ident": ident,
            }
        )
    return in_maps


def _erf(z):
    # Abramowitz-Stegun is not enough; use the exact erf from scipy if
    # present, else jax (available wherever the bass stack runs).
    try:
        from scipy.special import erf

        return erf(z)
    except ImportError:
        import jax

        with jax.default_device(jax.devices("cpu")[0]):
            return np.asarray(jax.scipy.special.erf(np.asarray(z, np.float32)))


def _refine(out, m8o, idxo, x, Key, init_mat, Value, w1, b1, w2, b2, tau=0.03):
    """Re-resolve tokens whose top-2 score gap is within tau (near-ties):
    recompute their argmin + output row in exact fp32 reference arithmetic."""
    f = np.float32
    Key = np.asarray(Key, f)
    V = np.asarray(init_mat, f) + np.asarray(Value, f)
    k2 = np.sum(Key * Key, axis=1)
    n_fixed = 0
    for c in range(out.shape[0]):
        m8 = m8o[c]  # [128, NT*8]
        m0 = m8[:, 0::8]  # [128, NT]
        m1 = m8[:, 1::8]
        gap = m0 - m1  # s-space gap == d2 second - d2 min
        dev_idx = idxo[c][:, 0::8].astype(np.int64)  # [128, NT]
        scale = 1.0 + np.abs(m0)
        flag = gap < tau * scale  # [128, NT]
        ps, ts = np.nonzero(flag)
        if ps.size == 0:
            continue
        toks = ts * P + ps
        xc = np.asarray(x[c], f)[toks]  # [F, 3]
        d2 = -2.0 * (xc @ Key.T) + k2[None, :]  # reference formula, fp32
        amin = np.argmin(d2, axis=1)
        mism = amin != dev_idx[ps, ts]
        if not np.any(mism):
            continue
        toks = toks[mism]
        amin = amin[mism]
        xe = np.asarray(x[c], f)[toks]
        rl = xe - Key[amin]
        pre = (rl @ np.asarray(w1, f) + np.asarray(b1, f)).astype(f)
        h = (0.5 * pre * (1.0 + _erf(pre / np.sqrt(f(2.0))))).astype(f)
        row = (h @ np.asarray(w2, f) + np.asarray(b2, f) + V[amin]).astype(f)
        out[c, toks, :] = row
        n_fixed += toks.size
    return n_fixed


def kernel(**inputs):
    from concourse.bass_utils import run_bass_kernel_spmd

    nc, names = _get_program()
    in_maps = _host_pack(**inputs)
    res = run_bass_kernel_spmd(nc, in_maps, core_ids=list(range(N_CORES)))

    out = np.zeros((B, N, D_OUT), np.float32)
    m8o = np.zeros((B, P, NT * 8), np.float32)
    idxo = np.zeros((B, P, NT * 8), np.uint32)
    for c in range(N_CORES):
        r = res.results[c]
        out[c] = r["outp"]
        m8o[c] = r["m8o"]
        idxo[c] = r["idxo"]

    _refine(out, m8o, idxo, **inputs)
    return out


if __name__ == "__main__":
    # smoke: build only
    _get_program()
    print("program built")
